# revision 1
# baseline (speedup 1.0000x reference)
"""Trainium2 Bass kernel for nn_L3_31799937859925 (sparse_attention).

Strategy:
- Each query row (label = seq_sort[j] in [0,64)) attends only to kv rows with
  emb_alloc == label, so we sort queries by label on the host and give each of
  the 8 cores a contiguous 2048-query slice (pure data parallel, no
  collectives). kv rows are label-sorted too, so each 512-query tile only needs
  a small contiguous kv window (W columns) + an additive -1e30 mask bias.
- On device everything is feature-major ([feature, query]) so no transposes are
  needed: scoresT = K'T @ x, softmax sums / rms stats via ones-column matmuls
  on the PE, per-query scalars broadcast across partitions via K=1 matmuls.
- norm_in_weight is folded into w_k, norm_out_weight into w_mix (host side).
- All heavy matmuls run in float32r (relaxed fp32, full PE rate, ~1.5e-4 rel).
"""
import numpy as np

import concourse.bass as bass
import concourse.tile as tile
from concourse import bacc, mybir
import concourse.bass_utils as bass_utils

F32 = mybir.dt.float32
F32R = mybir.dt.float32r
AF = mybir.ActivationFunctionType
MUL = mybir.AluOpType.mult
ADD = mybir.AluOpType.add

H, N_EMB, D_EMB, D_UP = 1024, 8192, 512, 2048
B, T = 4, 4096
BT = B * T                  # 16384
NC = 8                      # cores
NQ = BT // NC               # 2048 queries per core
QT = 512                    # queries per q-tile
NQT = NQ // QT              # 4 q-tiles per core
HC = H // 128               # 8
DC = D_EMB // 128           # 4
JC = D_UP // 128            # 16
KC = (D_UP + H) // 128      # 24 contraction chunks for mix
MC = H // 128               # 8 output chunks

LAST_RESULTS = None         # BassKernelResults of the most recent run (for test.py)
LAST_EXEC_S = None
_PROGRAM_CACHE = {}


def _build_program(W):
    """Build the SPMD single-core program. W = kv window width (mult of 128)."""
    n_kvc = W // 128
    nc = bacc.Bacc("TRN2", target_bir_lowering=False, debug=False,
                   enable_asserts=False)

    x_in = nc.dram_tensor("x_in", [128, HC, NQ], F32R, kind="ExternalInput")
    kt_in = nc.dram_tensor("kt_in", [NQT, 128, HC, W], F32R, kind="ExternalInput")
    v_in = nc.dram_tensor("v_in", [NQT, 128, n_kvc, D_EMB], F32R, kind="ExternalInput")
    b_in = nc.dram_tensor("b_in", [NQT, 128, n_kvc, QT], F32, kind="ExternalInput")
    wup_in = nc.dram_tensor("wup_in", [128, DC, D_UP], F32R, kind="ExternalInput")
    wmix_in = nc.dram_tensor("wmix_in", [MC, 128, KC, 128], F32R, kind="ExternalInput")
    out_d = nc.dram_tensor("out_d", [MC, 128, NQ], F32, kind="ExternalOutput")

    from contextlib import ExitStack
    with tile.TileContext(nc) as tc, ExitStack() as ctx:
        ec = ctx.enter_context
        cst = ec(tc.tile_pool(name="cst", bufs=1))
        pwup = ec(tc.tile_pool(name="wup", bufs=1))
        px = ec(tc.tile_pool(name="px", bufs=2))
        pkt = ec(tc.tile_pool(name="pkt", bufs=1))
        pv = ec(tc.tile_pool(name="pv", bufs=1))
        pb = ec(tc.tile_pool(name="pb", bufs=1))
        pwm = ec(tc.tile_pool(name="pwm", bufs=3))
        px2 = ec(tc.tile_pool(name="px2", bufs=2))
        ppu = ec(tc.tile_pool(name="ppu", bufs=1))
        pt = ec(tc.tile_pool(name="pt", bufs=3))
        pcomb = ec(tc.tile_pool(name="pcomb", bufs=1))
        pup = ec(tc.tile_pool(name="pup", bufs=1))
        pu2 = ec(tc.tile_pool(name="pu2", bufs=4))
        pbc = ec(tc.tile_pool(name="pbc", bufs=4))
        prows = ec(tc.tile_pool(name="prows", bufs=3))
        po = ec(tc.tile_pool(name="po", bufs=2))
        pbig = ec(tc.tile_pool(name="pbig", bufs=6, space="PSUM"))
        prow = ec(tc.tile_pool(name="prow", bufs=2, space="PSUM"))

        if True:
            ones_f = cst.tile([128, 1], F32)
            nc.vector.memset(ones_f, 1.0)
            ones_col = cst.tile([128, 1], F32R)
            nc.vector.tensor_copy(ones_col, ones_f)
            ones_rf = cst.tile([1, 128], F32)
            nc.vector.memset(ones_rf, 1.0)
            ones_row = cst.tile([1, 128], F32R)
            nc.vector.tensor_copy(ones_row, ones_rf)
            eps_t = cst.tile([128, 1], F32)
            nc.vector.memset(eps_t, 1e-6)

            wup_sb = pwup.tile([128, DC, D_UP], F32R)
            nc.sync.dma_start(wup_sb[:], wup_in.ap())

            for qt in range(NQT):
                qs = slice(qt * QT, (qt + 1) * QT)
                x_t = px.tile([128, HC, QT], F32R, tag="x")
                nc.sync.dma_start(x_t[:], x_in.ap()[:, :, qs])
                kt_t = pkt.tile([128, HC, W], F32R, tag="kt")
                nc.sync.dma_start(kt_t[:], kt_in.ap()[qt])
                v_t = pv.tile([128, n_kvc, D_EMB], F32R, tag="v")
                nc.sync.dma_start(v_t[:], v_in.ap()[qt])
                b_t = pb.tile([128, n_kvc, QT], F32, tag="b")
                nc.sync.dma_start(b_t[:], b_in.ap()[qt])

                # ---- rms_in stats: inv_rms per query as broadcast [128, QT]
                ss_ps = prow.tile([1, QT], F32, tag="row")
                for hc in range(HC):
                    x2 = px2.tile([128, QT], F32R, tag="x2")
                    nc.scalar.activation(x2, x_t[:, hc, :].bitcast(F32), AF.Square)
                    nc.tensor.matmul(ss_ps, lhsT=ones_col, rhs=x2,
                                     start=(hc == 0), stop=(hc == HC - 1))
                sd = prows.tile([1, QT], F32, tag="rows")
                nc.scalar.activation(sd, ss_ps, AF.Sqrt, bias=eps_t[:1],
                                     scale=1.0 / H)
                crf = prows.tile([1, QT], F32, tag="rows")
                nc.vector.reciprocal(crf, sd)
                cr = prows.tile([1, QT], F32R, tag="rowsr")
                nc.vector.tensor_copy(cr, crf)
                c_b = pbc.tile([128, QT], F32, tag="bc")

                # ---- scoresT [kv, q] per kv chunk; t = s*c + bias; pu = exp(t)
                pu_t = ppu.tile([128, n_kvc, QT], F32R, tag="pu")
                for kvc in range(n_kvc):
                    s_ps = pbig.tile([128, QT], F32, tag="big")
                    for hc in range(HC):
                        nc.tensor.matmul(
                            s_ps, lhsT=kt_t[:, hc, kvc * 128:(kvc + 1) * 128],
                            rhs=x_t[:, hc, :],
                            start=(hc == 0), stop=(hc == HC - 1))
                    if kvc == 0:
                        # emit bcast here so PE doesn't stall on the recip chain
                        cb_ps = pbig.tile([128, QT], F32, tag="big")
                        nc.tensor.matmul(cb_ps, lhsT=ones_row, rhs=cr,
                                         start=True, stop=True)
                        nc.vector.tensor_copy(c_b, cb_ps)
                    t_sb = pt.tile([128, QT], F32, tag="t")
                    nc.vector.tensor_tensor(t_sb, s_ps, c_b, MUL)
                    nc.vector.tensor_tensor(t_sb, t_sb, b_t[:, kvc, :], ADD)
                    nc.scalar.activation(pu_t[:, kvc, :], t_sb, AF.Exp)

                # ---- z = sum_kv pu ; z_b = 1/z broadcast
                z_ps = prow.tile([1, QT], F32, tag="row")
                for kvc in range(n_kvc):
                    nc.tensor.matmul(z_ps, lhsT=ones_col, rhs=pu_t[:, kvc, :],
                                     start=(kvc == 0), stop=(kvc == n_kvc - 1))
                zrf = prows.tile([1, QT], F32, tag="rows")
                nc.vector.reciprocal(zrf, z_ps)
                zr = prows.tile([1, QT], F32R, tag="rowsr")
                nc.vector.tensor_copy(zr, zrf)
                z_b = pbc.tile([128, QT], F32, tag="bc")

                # ---- combT [d, q] = V^T pu, normalized by z
                comb_t = pcomb.tile([128, DC, QT], F32R, tag="comb")
                for dc in range(DC):
                    c_ps = pbig.tile([128, QT], F32, tag="big")
                    for kvc in range(n_kvc):
                        nc.tensor.matmul(
                            c_ps, lhsT=v_t[:, kvc, dc * 128:(dc + 1) * 128],
                            rhs=pu_t[:, kvc, :],
                            start=(kvc == 0), stop=(kvc == n_kvc - 1))
                    if dc == 0:
                        zb_ps = pbig.tile([128, QT], F32, tag="big")
                        nc.tensor.matmul(zb_ps, lhsT=ones_row, rhs=zr,
                                         start=True, stop=True)
                        nc.vector.tensor_copy(z_b, zb_ps)
                    nc.vector.tensor_tensor(comb_t[:, dc, :], c_ps, z_b, MUL)

                # ---- upT [j, q] (raw, pre-norm) + sum of squares
                up_t = pup.tile([128, JC, QT], F32R, tag="up")
                ssu_ps = prow.tile([1, QT], F32, tag="row")
                pend = None
                for m in range(JC):
                    u_ps = pbig.tile([128, QT], F32, tag="big")
                    for dc in range(DC):
                        nc.tensor.matmul(
                            u_ps, lhsT=wup_sb[:, dc, m * 128:(m + 1) * 128],
                            rhs=comb_t[:, dc, :],
                            start=(dc == 0), stop=(dc == DC - 1))
                    if pend is not None:
                        nc.tensor.matmul(ssu_ps, lhsT=ones_col, rhs=pend,
                                         start=(m == 1), stop=False)
                    nc.vector.tensor_copy(up_t[:, m, :], u_ps)
                    u2 = pu2.tile([128, QT], F32R, tag="u2")
                    nc.scalar.activation(u2, u_ps, AF.Square)
                    pend = u2
                nc.tensor.matmul(ssu_ps, lhsT=ones_col, rhs=pend,
                                 start=False, stop=True)
                sdu = prows.tile([1, QT], F32, tag="rows")
                nc.scalar.activation(sdu, ssu_ps, AF.Sqrt, bias=eps_t[:1],
                                     scale=1.0 / D_UP)
                r2f = prows.tile([1, QT], F32, tag="rows")
                nc.vector.reciprocal(r2f, sdu)
                r2 = prows.tile([1, QT], F32R, tag="rowsr")
                nc.vector.tensor_copy(r2, r2f)
                i2_b = pbc.tile([128, QT], F32, tag="bc")

                # ---- mix: out[mc] = i2_b * (Wmix_up @ up) + (Wmix_x @ x)
                for mc in range(MC):
                    wm_t = pwm.tile([128, KC, 128], F32R, tag="wm")
                    nc.sync.dma_start(wm_t[:], wmix_in.ap()[mc])
                    a_ps = pbig.tile([128, QT], F32, tag="big")
                    for kc in range(JC):
                        nc.tensor.matmul(a_ps, lhsT=wm_t[:, kc, :],
                                         rhs=up_t[:, kc, :],
                                         start=(kc == 0), stop=(kc == JC - 1))
                    b_ps = pbig.tile([128, QT], F32, tag="big")
                    for kc in range(MC):
                        nc.tensor.matmul(b_ps, lhsT=wm_t[:, JC + kc, :],
                                         rhs=x_t[:, kc, :],
                                         start=(kc == 0), stop=(kc == MC - 1))
                    if mc == 0:
                        i2_ps = pbig.tile([128, QT], F32, tag="big")
                        nc.tensor.matmul(i2_ps, lhsT=ones_row, rhs=r2,
                                         start=True, stop=True)
                        nc.vector.tensor_copy(i2_b, i2_ps)
                    o_sb = po.tile([128, QT], F32, tag="o")
                    nc.vector.tensor_tensor(o_sb, a_ps, i2_b, MUL)
                    nc.vector.tensor_tensor(o_sb, o_sb, b_ps, ADD)
                    nc.sync.dma_start(out_d.ap()[mc][:, qs], o_sb[:])

    nc.compile()
    return nc


def _get_program(W):
    if W not in _PROGRAM_CACHE:
        _PROGRAM_CACHE[W] = _build_program(W)
    return _PROGRAM_CACHE[W]


def kernel(**inputs) -> np.ndarray:
    global LAST_RESULTS
    inp = np.asarray(inputs["input"], np.float32)
    fw = np.asarray(inputs["fw"]).astype(np.int64)
    seq_sort = np.asarray(inputs["seq_sort"]).astype(np.int64)
    keep_cols = np.asarray(inputs["keep_cols"]).astype(np.int64)
    emb_alloc = np.asarray(inputs["emb_alloc"]).astype(np.int64)
    starts = np.asarray(inputs["starts"]).astype(np.int64)
    ends = np.asarray(inputs["ends"]).astype(np.int64)
    bb = int(np.asarray(inputs["bb"]))
    w_k = np.asarray(inputs["w_k_weight"], np.float32)
    w_v = np.asarray(inputs["w_v_weight"], np.float32)
    w_up = np.asarray(inputs["w_up_weight"], np.float32)
    w_mix = np.asarray(inputs["w_mix_weight"], np.float32)
    w_in = np.asarray(inputs["norm_in_weight"], np.float32)
    w_out = np.asarray(inputs["norm_out_weight"], np.float32)

    x = inp.reshape(BT, H)
    nb = BT // bb
    st = starts.reshape(nb, bb).min(axis=1)
    en = ends.reshape(nb, bb).max(axis=1)

    # sort block-rows j by label (stable); row s of sorted space = block-row
    # order[s] = query fw[order[s]]
    order = np.argsort(seq_sort, kind="stable")
    perm = fw[order]                         # original flat query per sorted row
    lab_q = seq_sort[order]                  # label per sorted row
    blk_q = order // bb
    st_q = st[blk_q]
    en_q = en[blk_q]
    x_sorted = x[perm]                       # [BT, H]

    # kv side: keep + label-sort; fold norm_in into K
    la = emb_alloc[keep_cols]                # [M]
    M = la.shape[0]
    kv_order = np.argsort(la, kind="stable")
    la_s = la[kv_order]
    kvpos = kv_order                         # kept-position of sorted kv row
    Bm = (w_k[keep_cols] * w_in[None, :])[kv_order]   # [M, H]
    Cm = w_v[keep_cols][kv_order]            # [M, D_EMB]

    counts = np.bincount(la_s, minlength=64)
    gstart = np.concatenate([[0], np.cumsum(counts)])  # [65]

    # per-tile windows over sorted kv
    NT = BT // QT                            # 32 global q-tiles
    win = np.empty(NT, np.int64)
    need = 0
    for g in range(NT):
        l0 = lab_q[g * QT]
        l1 = lab_q[(g + 1) * QT - 1]
        win[g] = gstart[l0]
        need = max(need, gstart[l1 + 1] - gstart[l0])
    W = max(256, int(-(-need // 128) * 128))

    # padded kv arrays so windows never go OOB
    Mp = M + W
    Bm_p = np.zeros((Mp, H), np.float32); Bm_p[:M] = Bm
    Cm_p = np.zeros((Mp, D_EMB), np.float32); Cm_p[:M] = Cm
    la_p = np.full(Mp, -1, np.int64); la_p[:M] = la_s
    kvpos_p = np.full(Mp, -1, np.int64); kvpos_p[:M] = kvpos

    # mask bias per (sorted row, window col)
    kvi = win[:, None] + np.arange(W)[None, :]           # [NT, W]
    la_w = la_p[kvi]                                     # [NT, W]
    kp_w = kvpos_p[kvi]
    lab_t = lab_q.reshape(NT, QT)
    st_t = st_q.reshape(NT, QT)
    en_t = en_q.reshape(NT, QT)
    valid = ((la_w[:, None, :] == lab_t[:, :, None])
             & (kp_w[:, None, :] >= st_t[:, :, None])
             & (kp_w[:, None, :] < en_t[:, :, None]))    # [NT, QT, W]
    bias = np.where(valid, np.float32(0), np.float32(-1e30))

    KT_full = np.ascontiguousarray(Bm_p.T)               # [H, Mp]

    wm = w_mix.copy()
    wm[:, :D_UP] *= w_out[None, :]
    WmixT = np.ascontiguousarray(wm.T)                   # [3072, H]
    wmix_host = np.ascontiguousarray(
        WmixT.reshape(KC, 128, MC, 128).transpose(2, 1, 0, 3))  # [MC,128,KC,128]
    WupT = np.ascontiguousarray(w_up.T)                  # [D_EMB, D_UP]
    wup_host = np.ascontiguousarray(
        WupT.reshape(DC, 128, D_UP).transpose(1, 0, 2))  # [128, DC, D_UP]

    n_kvc = W // 128
    in_maps = []
    for c in range(NC):
        rows = slice(c * NQ, (c + 1) * NQ)
        x_c = np.ascontiguousarray(
            x_sorted[rows].T.reshape(HC, 128, NQ).transpose(1, 0, 2))  # [128,HC,NQ]
        kt_c = np.empty((NQT, 128, HC, W), np.float32)
        v_c = np.empty((NQT, 128, n_kvc, D_EMB), np.float32)
        b_c = np.empty((NQT, 128, n_kvc, QT), np.float32)
        for qt in range(NQT):
            g = c * NQT + qt
            w0 = win[g]
            kt_c[qt] = KT_full[:, w0:w0 + W].reshape(HC, 128, W).transpose(1, 0, 2)
            v_c[qt] = Cm_p[w0:w0 + W].reshape(n_kvc, 128, D_EMB).transpose(1, 0, 2)
            b_c[qt] = bias[g].T.reshape(n_kvc, 128, QT).transpose(1, 0, 2)
        in_maps.append({
            "x_in": x_c, "kt_in": kt_c, "v_in": v_c, "b_in": b_c,
            "wup_in": wup_host, "wmix_in": wmix_host,
        })

    nc = _get_program(W)
    import time as _time
    global LAST_EXEC_S
    _t0 = _time.time()
    LAST_RESULTS = bass_utils.run_bass_kernel_spmd(nc, in_maps,
                                                   core_ids=list(range(NC)))
    LAST_EXEC_S = _time.time() - _t0
    out_sorted = np.concatenate(
        [r["out_d"].transpose(2, 0, 1).reshape(NQ, H) for r in LAST_RESULTS.results],
        axis=0)                                          # [BT, H]
    final = np.empty((BT, H), np.float32)
    final[perm] = out_sorted
    return final.reshape(B, T, H)



# revision 5
# speedup vs baseline: 1.9534x; 1.9534x over previous
"""Trainium2 Bass kernel for nn_L3_31799937859925 (sparse_attention).

Strategy (v2 — fused-weights redesign):
- Queries sorted by label (host) -> 8 cores x 2048 queries, pure data parallel.
  kv rows label-sorted; each 512-query tile uses a contiguous kv window of W
  rows + additive -1e30 mask bias (covers label mismatch + st/en + padding).
- Key algebra: rms_out is a per-query SCALAR r2[q], so
    mix_up @ (w_out * rms(up)) = (mix_up * w_out) @ w_up @ comb * r2[q]
  with Wf = (w_mix[:, :d_up] * w_out) @ w_up  [h, d_emb] precomputed on host.
  ||up||^2 = ||L^T comb||^2 where L = chol(w_up^T w_up), and V folds into both:
    VL  = V @ L      (per kv row)  -> yraw = VL^T @ pu,  ssy = sum yraw^2
    VWf = V @ Wf^T   (per kv row)  -> A    = VWf^T @ (pu * zr*r2)
  so the device never materializes comb or up. out = A + Wmix_x @ x.
- All matmuls bf16 (PE full rate, tolerance 2e-2 >> bf16 error ~5e-3).
- Cross-partition stats (rms_in, softmax z, ssy) via gpsimd partition_all_reduce
  on the idle Pool engine at broadcast width [128,512]; no stats matmuls and no
  broadcast matmuls on the PE. PE does only real GEMM rows.
- Per out-chunk mc: B = Wmix_x@x accumulates first into PSUM, then A adds into
  the same bank (A depends on the late softmax/rms scale; B only on x).
"""
import numpy as np
import ml_dtypes

import concourse.bass as bass
import concourse.tile as tile
from concourse import bacc, mybir
import concourse.bass_utils as bass_utils
from concourse import bass_isa

F32 = mybir.dt.float32
BF16 = mybir.dt.bfloat16
AF = mybir.ActivationFunctionType
MUL = mybir.AluOpType.mult
ADD = mybir.AluOpType.add

H, N_EMB, D_EMB, D_UP = 1024, 8192, 512, 2048
B, T = 4, 4096
BT = B * T                  # 16384
NC = 8                      # cores
NQ = BT // NC               # 2048 queries per core
QT = 512                    # queries per q-tile
NQT = NQ // QT              # 4 q-tiles per core
HC = H // 128               # 8
MC = H // 128               # 8 output chunks
YC = D_EMB // 128           # 4 chunks of yraw

BF = np.dtype(ml_dtypes.bfloat16)

LAST_RESULTS = None         # BassKernelResults of the most recent run (for test.py)
LAST_EXEC_S = None
_PROGRAM_CACHE = {}


def _build_program(W):
    """SPMD single-core program. W = kv window width (multiple of 128)."""
    n_kvc = W // 128
    nc = bacc.Bacc("TRN2", target_bir_lowering=False, debug=False,
                   enable_asserts=False)

    x_in = nc.dram_tensor("x_in", [NQT, 128, HC, QT], BF16, kind="ExternalInput")
    kt_in = nc.dram_tensor("kt_in", [NQT, 128, HC, W], BF16, kind="ExternalInput")
    vl_in = nc.dram_tensor("vl_in", [NQT, 128, n_kvc, D_EMB], BF16, kind="ExternalInput")
    vw_in = nc.dram_tensor("vw_in", [NQT, 128, n_kvc, H], BF16, kind="ExternalInput")
    b_in = nc.dram_tensor("b_in", [NQT, 128, n_kvc, QT], BF16, kind="ExternalInput")
    wmx_in = nc.dram_tensor("wmx_in", [MC, 128, HC, 128], BF16, kind="ExternalInput")
    out_d = nc.dram_tensor("out_d", [MC, 128, NQ], F32, kind="ExternalOutput")

    from contextlib import ExitStack
    with tile.TileContext(nc) as tc, ExitStack() as ctx:
        ec = ctx.enter_context
        pwmx = ec(tc.tile_pool(name="wmx", bufs=1))
        px = ec(tc.tile_pool(name="px", bufs=2))
        pkt = ec(tc.tile_pool(name="pkt", bufs=2))
        pvl = ec(tc.tile_pool(name="pvl", bufs=2))
        pvw = ec(tc.tile_pool(name="pvw", bufs=2))
        pb = ec(tc.tile_pool(name="pb", bufs=2))
        px2 = ec(tc.tile_pool(name="px2", bufs=4))
        pacc = ec(tc.tile_pool(name="pacc", bufs=2))
        ptmp = ec(tc.tile_pool(name="ptmp", bufs=4))
        prow = ec(tc.tile_pool(name="prow", bufs=4))
        pcb = ec(tc.tile_pool(name="pcb", bufs=2))
        pscl = ec(tc.tile_pool(name="pscl", bufs=2))
        pt = ec(tc.tile_pool(name="pt", bufs=2))
        ppu = ec(tc.tile_pool(name="ppu", bufs=2))
        ppus = ec(tc.tile_pool(name="ppus", bufs=2))
        pysq = ec(tc.tile_pool(name="pysq", bufs=3))
        po = ec(tc.tile_pool(name="po", bufs=3))
        psc = ec(tc.tile_pool(name="psc", bufs=2, space="PSUM"))
        pyp = ec(tc.tile_pool(name="pyp", bufs=2, space="PSUM"))
        pop = ec(tc.tile_pool(name="pop", bufs=3, space="PSUM"))

        cst = ec(tc.tile_pool(name="cst", bufs=1))
        eps_t = cst.tile([128, 1], F32)
        nc.vector.memset(eps_t, 1e-6)

        # resident Wmix_x, loaded in MC chunks so B(0) can start early
        wmx_sb = pwmx.tile([128, HC, H], BF16)
        for mc in range(MC):
            nc.sync.dma_start(wmx_sb[:, :, mc * 128:(mc + 1) * 128],
                              wmx_in.ap()[mc])

        for qt in range(NQT):
            qs = slice(qt * QT, (qt + 1) * QT)
            x_t = px.tile([128, HC, QT], BF16, tag="x")
            nc.sync.dma_start(x_t[:], x_in.ap()[qt])
            kt_t = pkt.tile([128, HC, W], BF16, tag="kt")
            nc.sync.dma_start(kt_t[:], kt_in.ap()[qt])
            vl_t = pvl.tile([128, n_kvc, D_EMB], BF16, tag="vl")
            nc.sync.dma_start(vl_t[:], vl_in.ap()[qt])
            vw_t = pvw.tile([128, n_kvc, H], BF16, tag="vw")
            nc.sync.dma_start(vw_t[:], vw_in.ap()[qt])
            b_t = pb.tile([128, n_kvc, QT], BF16, tag="b")
            nc.sync.dma_start(b_t[:], b_in.ap()[qt])

            # ---- rms_in: c_b[p,q] = 1/sqrt(mean_h x^2 + eps)  (all partitions)
            ssx = pacc.tile([128, QT], F32, tag="acc")
            for hc in range(HC):
                x2 = px2.tile([128, QT], F32, tag="x2")
                nc.scalar.activation(x2, x_t[:, hc, :], AF.Square)
                if hc == 0:
                    first_x2 = x2
                elif hc == 1:
                    nc.gpsimd.tensor_tensor(ssx, first_x2, x2, ADD)
                else:
                    nc.gpsimd.tensor_tensor(ssx, ssx, x2, ADD)
            ssb = ptmp.tile([128, QT], F32, tag="tmp")
            nc.gpsimd.partition_all_reduce(ssb, ssx, 128, bass_isa.ReduceOp.add)
            sd = ptmp.tile([128, QT], F32, tag="tmp")
            nc.scalar.activation(sd, ssb, AF.Sqrt, bias=eps_t, scale=1.0 / H)
            c_b = pcb.tile([128, QT], F32, tag="cb")
            nc.vector.reciprocal(c_b, sd)

            # ---- scoresT chunks -> t = s*c + bias -> pu = exp(t) (bf16)
            t_sb = pt.tile([128, n_kvc, QT], F32, tag="t")
            pu_t = ppu.tile([128, n_kvc, QT], BF16, tag="pu")
            for kvc in range(n_kvc):
                s_ps = psc.tile([128, QT], F32, tag="sc")
                for hc in range(HC):
                    nc.tensor.matmul(
                        s_ps, lhsT=kt_t[:, hc, kvc * 128:(kvc + 1) * 128],
                        rhs=x_t[:, hc, :],
                        start=(hc == 0), stop=(hc == HC - 1))
                nc.vector.tensor_tensor(t_sb[:, kvc, :], s_ps, c_b, MUL)
            nc.vector.tensor_tensor(t_sb[:], t_sb[:], b_t[:], ADD)
            nc.scalar.activation(pu_t[:], t_sb[:], AF.Exp)

            # ---- z = sum_kv pu (Pool all-reduce); zr = 1/z
            if n_kvc == 2:
                zacc = pacc.tile([128, QT], F32, tag="acc")
                nc.gpsimd.tensor_tensor(zacc, pu_t[:, 0, :], pu_t[:, 1, :], ADD)
            else:
                zacc = pacc.tile([128, QT], F32, tag="acc")
                nc.gpsimd.tensor_tensor(zacc, pu_t[:, 0, :], pu_t[:, 1, :], ADD)
                for kvc in range(2, n_kvc):
                    nc.gpsimd.tensor_tensor(zacc, zacc, pu_t[:, kvc, :], ADD)
            z_b = ptmp.tile([128, QT], F32, tag="tmp")
            nc.gpsimd.partition_all_reduce(z_b, zacc, 128, bass_isa.ReduceOp.add)
            zr = prow.tile([128, QT], F32, tag="row")
            nc.vector.reciprocal(zr, z_b)

            # ---- first B chunk early (covers exp/z latency)
            o_ps0 = pop.tile([128, QT], F32, tag="op")
            for hc in range(HC):
                nc.tensor.matmul(o_ps0, lhsT=wmx_sb[:, hc, 0:128],
                                 rhs=x_t[:, hc, :],
                                 start=(hc == 0), stop=False)

            # ---- yraw = VL^T @ pu ; ssy = allreduce(sum yraw^2)
            yacc = pacc.tile([128, QT], F32, tag="acc")
            for yc in range(YC):
                y_ps = pyp.tile([128, QT], F32, tag="yp")
                for kvc in range(n_kvc):
                    nc.tensor.matmul(
                        y_ps, lhsT=vl_t[:, kvc, yc * 128:(yc + 1) * 128],
                        rhs=pu_t[:, kvc, :],
                        start=(kvc == 0), stop=(kvc == n_kvc - 1))
                ysq = pysq.tile([128, QT], F32, tag="ysq")
                nc.scalar.activation(ysq, y_ps, AF.Square)
                if yc == 0:
                    first_ysq = ysq
                elif yc == 1:
                    nc.gpsimd.tensor_tensor(yacc, first_ysq, ysq, ADD)
                else:
                    nc.gpsimd.tensor_tensor(yacc, yacc, ysq, ADD)
            ssy = ptmp.tile([128, QT], F32, tag="tmp")
            nc.gpsimd.partition_all_reduce(ssy, yacc, 128, bass_isa.ReduceOp.add)

            # ---- scale = zr * r2 ; r2 = 1/sqrt(zr^2*ssy/D_UP + eps)
            t1 = prow.tile([128, QT], F32, tag="row")
            nc.vector.tensor_tensor(t1, zr, zr, MUL)
            t2 = prow.tile([128, QT], F32, tag="row")
            nc.vector.tensor_tensor(t2, t1, ssy, MUL)
            sd2 = prow.tile([128, QT], F32, tag="row")
            nc.scalar.activation(sd2, t2, AF.Sqrt, bias=eps_t, scale=1.0 / D_UP)
            r2 = prow.tile([128, QT], F32, tag="row")
            nc.vector.reciprocal(r2, sd2)
            sclf = prow.tile([128, QT], F32, tag="row")
            nc.vector.tensor_tensor(sclf, zr, r2, MUL)
            scl = pscl.tile([128, QT], BF16, tag="scl")
            nc.vector.tensor_copy(scl, sclf)

            # ---- pu_s = pu * scale (bf16, 2x DVE)
            pus_t = ppus.tile([128, n_kvc, QT], BF16, tag="pus")
            for kvc in range(n_kvc):
                nc.vector.tensor_tensor(pus_t[:, kvc, :], pu_t[:, kvc, :],
                                        scl, MUL)

            # ---- out[mc] = B (Wmix_x @ x) + A (VWf^T @ pu_s); B first in bank
            o_prev = o_ps0
            for mc in range(MC):
                if mc + 1 < MC:
                    o_next = pop.tile([128, QT], F32, tag="op")
                    for hc in range(HC):
                        nc.tensor.matmul(
                            o_next, lhsT=wmx_sb[:, hc,
                                               (mc + 1) * 128:(mc + 2) * 128],
                            rhs=x_t[:, hc, :],
                            start=(hc == 0), stop=False)
                for kvc in range(n_kvc):
                    nc.tensor.matmul(
                        o_prev, lhsT=vw_t[:, kvc, mc * 128:(mc + 1) * 128],
                        rhs=pus_t[:, kvc, :],
                        start=False, stop=(kvc == n_kvc - 1))
                o_sb = po.tile([128, QT], F32, tag="o")
                nc.vector.tensor_copy(o_sb, o_prev)
                nc.sync.dma_start(out_d.ap()[mc][:, qs], o_sb[:])
                if mc + 1 < MC:
                    o_prev = o_next

    nc.compile()
    return nc


def _get_program(W):
    if W not in _PROGRAM_CACHE:
        _PROGRAM_CACHE[W] = _build_program(W)
    return _PROGRAM_CACHE[W]


def kernel(**inputs) -> np.ndarray:
    global LAST_RESULTS
    inp = np.asarray(inputs["input"], np.float32)
    fw = np.asarray(inputs["fw"]).astype(np.int64)
    seq_sort = np.asarray(inputs["seq_sort"]).astype(np.int64)
    keep_cols = np.asarray(inputs["keep_cols"]).astype(np.int64)
    emb_alloc = np.asarray(inputs["emb_alloc"]).astype(np.int64)
    starts = np.asarray(inputs["starts"]).astype(np.int64)
    ends = np.asarray(inputs["ends"]).astype(np.int64)
    bb = int(np.asarray(inputs["bb"]))
    w_k = np.asarray(inputs["w_k_weight"], np.float32)
    w_v = np.asarray(inputs["w_v_weight"], np.float32)
    w_up = np.asarray(inputs["w_up_weight"], np.float32)
    w_mix = np.asarray(inputs["w_mix_weight"], np.float32)
    w_in = np.asarray(inputs["norm_in_weight"], np.float32)
    w_out = np.asarray(inputs["norm_out_weight"], np.float32)

    x = inp.reshape(BT, H)
    nb = BT // bb
    st = starts.reshape(nb, bb).min(axis=1)
    en = ends.reshape(nb, bb).max(axis=1)

    # sort queries by label (stable); sorted row s <- original flat query perm[s]
    order = np.argsort(seq_sort, kind="stable")
    perm = fw[order]
    lab_q = seq_sort[order]
    blk_q = order // bb
    st_q = st[blk_q]
    en_q = en[blk_q]
    x_sorted = x[perm]                       # [BT, H]

    # kv side: keep + label-sort; fold norm_in into K
    la = emb_alloc[keep_cols]                # [M]
    M = la.shape[0]
    kv_order = np.argsort(la, kind="stable")
    la_s = la[kv_order]
    kvpos = kv_order
    Bm = (w_k[keep_cols] * w_in[None, :])[kv_order]   # [M, H]
    Cm = w_v[keep_cols][kv_order].astype(np.float64)  # [M, D_EMB]

    counts = np.bincount(la_s, minlength=64)
    gstart = np.concatenate([[0], np.cumsum(counts)])

    # fused weights
    Wf = (w_mix[:, :D_UP] * w_out[None, :]).astype(np.float64) @ w_up.astype(np.float64)  # [H, D_EMB]
    Mq = w_up.astype(np.float64).T @ w_up.astype(np.float64)      # [D_EMB, D_EMB]
    L = np.linalg.cholesky(Mq)                                    # M = L L^T
    VL = (Cm @ L).astype(np.float32)                              # [M, D_EMB]
    VWf = (Cm @ Wf.T).astype(np.float32)                          # [M, H]

    # per-tile windows over sorted kv
    NT = BT // QT
    win = np.empty(NT, np.int64)
    need = 0
    for g in range(NT):
        l0 = lab_q[g * QT]
        l1 = lab_q[(g + 1) * QT - 1]
        win[g] = gstart[l0]
        need = max(need, gstart[l1 + 1] - gstart[l0])
    W = max(256, int(-(-need // 128) * 128))

    # padded kv arrays so windows never go OOB
    Mp = M + W
    KT_p = np.zeros((H, Mp), np.float32)
    KT_p[:, :M] = Bm.T
    VL_p = np.zeros((Mp, D_EMB), np.float32); VL_p[:M] = VL
    VW_p = np.zeros((Mp, H), np.float32); VW_p[:M] = VWf
    la_p = np.full(Mp, -1, np.int64); la_p[:M] = la_s
    kvpos_p = np.full(Mp, -1, np.int64); kvpos_p[:M] = kvpos

    # mask bias per (window col, sorted row)
    kvi = win[:, None] + np.arange(W)[None, :]           # [NT, W]
    la_w = la_p[kvi]
    kp_w = kvpos_p[kvi]
    lab_t = lab_q.reshape(NT, QT)
    st_t = st_q.reshape(NT, QT)
    en_t = en_q.reshape(NT, QT)
    valid = ((la_w[:, None, :] == lab_t[:, :, None])
             & (kp_w[:, None, :] >= st_t[:, :, None])
             & (kp_w[:, None, :] < en_t[:, :, None]))    # [NT, QT, W]
    bias = np.where(valid, np.float32(0), np.float32(-1e30))

    Wm_x = w_mix[:, D_UP:]                               # [H, H]
    wmx_host = np.ascontiguousarray(
        Wm_x.T.reshape(HC, 128, H).transpose(1, 0, 2)    # [128, HC, H]
        .reshape(128, HC, MC, 128).transpose(2, 0, 1, 3)).astype(BF)  # [MC,128,HC,128]

    n_kvc = W // 128
    in_maps = []
    for c in range(NC):
        x_c = np.empty((NQT, 128, HC, QT), BF)
        kt_c = np.empty((NQT, 128, HC, W), BF)
        vl_c = np.empty((NQT, 128, n_kvc, D_EMB), BF)
        vw_c = np.empty((NQT, 128, n_kvc, H), BF)
        b_c = np.empty((NQT, 128, n_kvc, QT), BF)
        for qt in range(NQT):
            g = c * NQT + qt
            rows = slice(g * QT, (g + 1) * QT)
            w0 = win[g]
            x_c[qt] = x_sorted[rows].T.reshape(HC, 128, QT).transpose(1, 0, 2)
            kt_c[qt] = KT_p[:, w0:w0 + W].reshape(HC, 128, W).transpose(1, 0, 2)
            vl_c[qt] = VL_p[w0:w0 + W].reshape(n_kvc, 128, D_EMB).transpose(1, 0, 2)
            vw_c[qt] = VW_p[w0:w0 + W].reshape(n_kvc, 128, H).transpose(1, 0, 2)
            b_c[qt] = bias[g].T.reshape(n_kvc, 128, QT).transpose(1, 0, 2)
        in_maps.append({
            "x_in": x_c, "kt_in": kt_c, "vl_in": vl_c, "vw_in": vw_c,
            "b_in": b_c, "wmx_in": wmx_host,
        })

    nc = _get_program(W)
    import time as _time
    global LAST_EXEC_S
    _t0 = _time.time()
    LAST_RESULTS = bass_utils.run_bass_kernel_spmd(nc, in_maps,
                                                   core_ids=list(range(NC)))
    LAST_EXEC_S = _time.time() - _t0
    out_sorted = np.concatenate(
        [r["out_d"].transpose(2, 0, 1).reshape(NQ, H) for r in LAST_RESULTS.results],
        axis=0)                                          # [BT, H]
    final = np.empty((BT, H), np.float32)
    final[perm] = out_sorted
    return final.reshape(B, T, H)


# revision 10
# speedup vs baseline: 2.5985x; 1.3302x over previous
"""Trainium2 Bass kernel for nn_L3_31799937859925 (sparse_attention).

Strategy (v2 — fused-weights redesign):
- Queries sorted by label (host) -> 8 cores x 2048 queries, pure data parallel.
  kv rows label-sorted; each 512-query tile uses a contiguous kv window of W
  rows + additive -1e30 mask bias (covers label mismatch + st/en + padding).
- Key algebra: rms_out is a per-query SCALAR r2[q], so
    mix_up @ (w_out * rms(up)) = (mix_up * w_out) @ w_up @ comb * r2[q]
  with Wf = (w_mix[:, :d_up] * w_out) @ w_up  [h, d_emb] precomputed on host.
  ||up||^2 = ||L^T comb||^2 where L = chol(w_up^T w_up), and V folds into both:
    VL  = V @ L      (per kv row)  -> yraw = VL^T @ pu,  ssy = sum yraw^2
    VWf = V @ Wf^T   (per kv row)  -> A    = VWf^T @ (pu * zr*r2)
  so the device never materializes comb or up. out = A + Wmix_x @ x.
- All matmuls bf16 (PE full rate, tolerance 2e-2 >> bf16 error ~5e-3).
- Cross-partition stats (rms_in, softmax z, ssy) via gpsimd partition_all_reduce
  on the idle Pool engine at broadcast width [128,512]; no stats matmuls and no
  broadcast matmuls on the PE. PE does only real GEMM rows.
- Per out-chunk mc: B = Wmix_x@x accumulates first into PSUM, then A adds into
  the same bank (A depends on the late softmax/rms scale; B only on x).
"""
import numpy as np
import ml_dtypes

import concourse.bass as bass
import concourse.tile as tile
from concourse import bacc, mybir
import concourse.bass_utils as bass_utils
from concourse import bass_isa

F32 = mybir.dt.float32
BF16 = mybir.dt.bfloat16
AF = mybir.ActivationFunctionType
MUL = mybir.AluOpType.mult
ADD = mybir.AluOpType.add

H, N_EMB, D_EMB, D_UP = 1024, 8192, 512, 2048
B, T = 4, 4096
BT = B * T                  # 16384
NC = 8                      # cores
NQ = BT // NC               # 2048 queries per core
QT = 512                    # queries per q-tile
NQT = NQ // QT              # 4 q-tiles per core
HC = H // 128               # 8
MC = H // 128               # 8 output chunks
YC = D_EMB // 128           # 4 chunks of yraw

BF = np.dtype(ml_dtypes.bfloat16)

LAST_RESULTS = None         # BassKernelResults of the most recent run (for test.py)
LAST_EXEC_S = None
_PROGRAM_CACHE = {}


def _build_program(W):
    """SPMD single-core program. W = kv window width (multiple of 128)."""
    n_kvc = W // 128
    nc = bacc.Bacc("TRN2", target_bir_lowering=False, debug=False,
                   enable_asserts=False)

    x_in = nc.dram_tensor("x_in", [NQT, 128, HC, QT], BF16, kind="ExternalInput")
    kt_in = nc.dram_tensor("kt_in", [NQT, 128, HC, W], BF16, kind="ExternalInput")
    vl_in = nc.dram_tensor("vl_in", [NQT, 128, n_kvc, D_EMB], BF16, kind="ExternalInput")
    vw_in = nc.dram_tensor("vw_in", [NQT, 128, n_kvc, H], BF16, kind="ExternalInput")
    b_in = nc.dram_tensor("b_in", [NQT, 128, n_kvc, QT], BF16, kind="ExternalInput")
    wmx_in = nc.dram_tensor("wmx_in", [MC, 128, HC * 128], BF16, kind="ExternalInput")
    out_d = nc.dram_tensor("out_d", [MC, 128, NQ], BF16, kind="ExternalOutput")

    from contextlib import ExitStack
    with tile.TileContext(nc) as tc, ExitStack() as ctx:
        ec = ctx.enter_context
        cst = ec(tc.tile_pool(name="cst", bufs=1))
        pwmx = ec(tc.tile_pool(name="wmx", bufs=1))
        px = ec(tc.tile_pool(name="px", bufs=3))
        pkt = ec(tc.tile_pool(name="pkt", bufs=3))
        pvl = ec(tc.tile_pool(name="pvl", bufs=3))
        pvw = ec(tc.tile_pool(name="pvw", bufs=3))
        pb = ec(tc.tile_pool(name="pb", bufs=3))
        px2 = ec(tc.tile_pool(name="px2", bufs=4))
        pps = ec(tc.tile_pool(name="pps", bufs=8))
        pacc = ec(tc.tile_pool(name="pacc", bufs=2))
        ptmp = ec(tc.tile_pool(name="ptmp", bufs=4))
        prow = ec(tc.tile_pool(name="prow", bufs=4))
        pcb = ec(tc.tile_pool(name="pcb", bufs=2))
        pscl = ec(tc.tile_pool(name="pscl", bufs=2))
        pt = ec(tc.tile_pool(name="pt", bufs=2))
        ppu = ec(tc.tile_pool(name="ppu", bufs=2))
        ppus = ec(tc.tile_pool(name="ppus", bufs=2))
        pysq = ec(tc.tile_pool(name="pysq", bufs=4))
        po = ec(tc.tile_pool(name="po", bufs=4))
        psc = ec(tc.tile_pool(name="psc", bufs=2, space="PSUM"))
        pyp = ec(tc.tile_pool(name="pyp", bufs=2, space="PSUM"))
        pop = ec(tc.tile_pool(name="pop", bufs=4, space="PSUM"))

        eps_t = cst.tile([128, 1], F32)
        nc.vector.memset(eps_t, 1e-6)

        x_ts = [None] * NQT
        kt_ts = [None] * NQT
        vl_ts = [None] * NQT
        vw_ts = [None] * NQT
        b_ts = [None] * NQT

        def emit_loads(qt):
            x_t = px.tile([128, HC, QT], BF16, tag="x")
            nc.sync.dma_start(x_t[:, 0:4, :], x_in.ap()[qt][:, 0:4, :])
            nc.sync.dma_start(x_t[:, 4:8, :], x_in.ap()[qt][:, 4:8, :])
            kt_t = pkt.tile([128, HC, W], BF16, tag="kt")
            nc.sync.dma_start(kt_t[:, 0:4, :], kt_in.ap()[qt][:, 0:4, :])
            nc.sync.dma_start(kt_t[:, 4:8, :], kt_in.ap()[qt][:, 4:8, :])
            vl_t = pvl.tile([128, n_kvc, D_EMB], BF16, tag="vl")
            nc.sync.dma_start(vl_t[:], vl_in.ap()[qt])
            vw_t = pvw.tile([128, n_kvc, H], BF16, tag="vw")
            nc.sync.dma_start(vw_t[:], vw_in.ap()[qt])
            b_t = pb.tile([128, n_kvc, QT], BF16, tag="b")
            nc.sync.dma_start(b_t[:], b_in.ap()[qt])
            x_ts[qt], kt_ts[qt], vl_ts[qt] = x_t, kt_t, vl_t
            vw_ts[qt], b_ts[qt] = vw_t, b_t

        cbs = [None] * NQT

        def emit_stats(qt):
            """rms_in chain for tile qt -> c_b (all partitions broadcast)."""
            x_t = x_ts[qt]
            x2s = []
            for hc in range(HC):
                x2 = px2.tile([128, QT], F32, tag="x2")
                nc.scalar.activation(x2, x_t[:, hc, :], AF.Square)
                x2s.append(x2)
            parts = []
            for i in range(4):
                p = pps.tile([128, QT], F32, tag="ps")
                eng = nc.gpsimd if i % 2 == 0 else nc.vector
                eng.tensor_tensor(p, x2s[2 * i], x2s[2 * i + 1], ADD)
                parts.append(p)
            q0 = pps.tile([128, QT], F32, tag="ps")
            nc.gpsimd.tensor_tensor(q0, parts[0], parts[1], ADD)
            q1 = pps.tile([128, QT], F32, tag="ps")
            nc.vector.tensor_tensor(q1, parts[2], parts[3], ADD)
            ssx = pacc.tile([128, QT], F32, tag="acc")
            nc.gpsimd.tensor_tensor(ssx, q0, q1, ADD)
            ssb = ptmp.tile([128, QT], F32, tag="tmp")
            nc.gpsimd.partition_all_reduce(ssb, ssx, 128, bass_isa.ReduceOp.add)
            sd = ptmp.tile([128, QT], F32, tag="tmp")
            nc.scalar.activation(sd, ssb, AF.Sqrt, bias=eps_t, scale=1.0 / H)
            c_b = pcb.tile([128, QT], F32, tag="cb")
            nc.vector.reciprocal(c_b, sd)
            cbs[qt] = c_b

        # ---- prologue: tile0+tile1 loads, resident wmx, tile0 stats
        emit_loads(0)
        wmx_sb = pwmx.tile([128, MC, HC * 128], BF16)
        for mc in range(MC):
            nc.sync.dma_start(wmx_sb[:, mc, :], wmx_in.ap()[mc])
        emit_loads(1)
        emit_stats(0)

        for qt in range(NQT):
            qs = slice(qt * QT, (qt + 1) * QT)
            if qt + 2 < NQT:
                emit_loads(qt + 2)
            x_t, kt_t, vl_t = x_ts[qt], kt_ts[qt], vl_ts[qt]
            vw_t, b_t = vw_ts[qt], b_ts[qt]
            c_b = cbs[qt]

            # ---- scoresT chunks -> t = s*c + bias -> pu = exp(t), per chunk
            pu_t = ppu.tile([128, n_kvc, QT], BF16, tag="pu")
            for kvc in range(n_kvc):
                s_ps = psc.tile([128, QT], F32, tag="sc")
                for hc in range(HC):
                    nc.tensor.matmul(
                        s_ps, lhsT=kt_t[:, hc, kvc * 128:(kvc + 1) * 128],
                        rhs=x_t[:, hc, :],
                        start=(hc == 0), stop=(hc == HC - 1))
                t_sb = pt.tile([128, QT], F32, tag="t")
                nc.vector.tensor_tensor(t_sb, s_ps, c_b, MUL)
                nc.vector.tensor_tensor(t_sb, t_sb, b_t[:, kvc, :], ADD)
                nc.scalar.activation(pu_t[:, kvc, :], t_sb, AF.Exp)

            # ---- z = sum_kv pu (Pool all-reduce); zr = 1/z
            zacc = pacc.tile([128, QT], F32, tag="acc")
            nc.gpsimd.tensor_tensor(zacc, pu_t[:, 0, :], pu_t[:, 1, :], ADD)
            for kvc in range(2, n_kvc):
                nc.gpsimd.tensor_tensor(zacc, zacc, pu_t[:, kvc, :], ADD)
            z_b = ptmp.tile([128, QT], F32, tag="tmp")
            nc.gpsimd.partition_all_reduce(z_b, zacc, 128, bass_isa.ReduceOp.add)
            zse = prow.tile([128, QT], F32, tag="row")
            nc.vector.scalar_tensor_tensor(zse, z_b, 1e-6, z_b, MUL, MUL)

            # ---- early B chunks (cover exp/z/scale latency); deeper on tile 0
            LA = 4
            o_tiles = {}
            def emit_B(mc):
                o_ps = pop.tile([128, QT], F32, tag="op")
                for hc in range(HC):
                    nc.tensor.matmul(
                        o_ps, lhsT=wmx_sb[:, mc, hc * 128:(hc + 1) * 128],
                        rhs=x_t[:, hc, :],
                        start=(hc == 0), stop=False)
                o_tiles[mc] = o_ps
            emit_B(0)

            # ---- yraw = VL^T @ pu ; ssy = allreduce(sum yraw^2)
            ysqs = []
            for yc in range(YC):
                y_ps = pyp.tile([128, QT], F32, tag="yp")
                for kvc in range(n_kvc):
                    nc.tensor.matmul(
                        y_ps, lhsT=vl_t[:, kvc, yc * 128:(yc + 1) * 128],
                        rhs=pu_t[:, kvc, :],
                        start=(kvc == 0), stop=(kvc == n_kvc - 1))
                ysq = pysq.tile([128, QT], F32, tag="ysq")
                nc.scalar.activation(ysq, y_ps, AF.Square)
                ysqs.append(ysq)
            for mc in range(1, LA):
                emit_B(mc)
            ya0 = pps.tile([128, QT], F32, tag="ps")
            nc.gpsimd.tensor_tensor(ya0, ysqs[0], ysqs[1], ADD)
            ya1 = pps.tile([128, QT], F32, tag="ps")
            nc.vector.tensor_tensor(ya1, ysqs[2], ysqs[3], ADD)
            yacc = pacc.tile([128, QT], F32, tag="acc")
            nc.gpsimd.tensor_tensor(yacc, ya0, ya1, ADD)
            ssy = ptmp.tile([128, QT], F32, tag="tmp")
            nc.gpsimd.partition_all_reduce(ssy, yacc, 128, bass_isa.ReduceOp.add)

            # ---- scale = zr*r2 = 1/sqrt(ssy/D_UP + eps*z^2)  (exact algebra)
            u = prow.tile([128, QT], F32, tag="row")
            nc.vector.scalar_tensor_tensor(u, ssy, 1.0 / D_UP, zse, MUL, ADD)
            sd2 = prow.tile([128, QT], F32, tag="row")
            nc.scalar.activation(sd2, u, AF.Sqrt)
            scl = pscl.tile([128, QT], BF16, tag="scl")
            with nc.allow_low_precision(reason="softmax/rms scale to bf16; tol 2e-2"):
                nc.vector.reciprocal(scl, sd2)

            # ---- next tile's rms_in chain (fills Act/Pool/DVE slack)
            if qt + 1 < NQT:
                emit_stats(qt + 1)

            # ---- pu_s = pu * scale (bf16, 2x DVE)
            pus_t = ppus.tile([128, n_kvc, QT], BF16, tag="pus")
            for kvc in range(n_kvc):
                nc.vector.tensor_tensor(pus_t[:, kvc, :], pu_t[:, kvc, :],
                                        scl, MUL)

            # ---- out[mc] = B + A; A adds into B's psum bank; copy+DMA out
            for mc in range(MC):
                if mc + LA < MC:
                    emit_B(mc + LA)
                o_ps = o_tiles.pop(mc)
                for kvc in range(n_kvc):
                    nc.tensor.matmul(
                        o_ps, lhsT=vw_t[:, kvc, mc * 128:(mc + 1) * 128],
                        rhs=pus_t[:, kvc, :],
                        start=False, stop=(kvc == n_kvc - 1))
                o_sb = po.tile([128, QT], BF16, tag="o")
                nc.vector.tensor_copy(o_sb, o_ps)
                nc.scalar.dma_start(out_d.ap()[mc][:, qs], o_sb[:])

    nc.compile()
    return nc


def _get_program(W):
    if W not in _PROGRAM_CACHE:
        _PROGRAM_CACHE[W] = _build_program(W)
    return _PROGRAM_CACHE[W]


def kernel(**inputs) -> np.ndarray:
    global LAST_RESULTS
    inp = np.asarray(inputs["input"], np.float32)
    fw = np.asarray(inputs["fw"]).astype(np.int64)
    seq_sort = np.asarray(inputs["seq_sort"]).astype(np.int64)
    keep_cols = np.asarray(inputs["keep_cols"]).astype(np.int64)
    emb_alloc = np.asarray(inputs["emb_alloc"]).astype(np.int64)
    starts = np.asarray(inputs["starts"]).astype(np.int64)
    ends = np.asarray(inputs["ends"]).astype(np.int64)
    bb = int(np.asarray(inputs["bb"]))
    w_k = np.asarray(inputs["w_k_weight"], np.float32)
    w_v = np.asarray(inputs["w_v_weight"], np.float32)
    w_up = np.asarray(inputs["w_up_weight"], np.float32)
    w_mix = np.asarray(inputs["w_mix_weight"], np.float32)
    w_in = np.asarray(inputs["norm_in_weight"], np.float32)
    w_out = np.asarray(inputs["norm_out_weight"], np.float32)

    x = inp.reshape(BT, H)
    nb = BT // bb
    st = starts.reshape(nb, bb).min(axis=1)
    en = ends.reshape(nb, bb).max(axis=1)

    # sort queries by label (stable); sorted row s <- original flat query perm[s]
    order = np.argsort(seq_sort, kind="stable")
    perm = fw[order]
    lab_q = seq_sort[order]
    blk_q = order // bb
    st_q = st[blk_q]
    en_q = en[blk_q]
    x_sorted = x[perm]                       # [BT, H]

    # kv side: keep + label-sort; fold norm_in into K
    la = emb_alloc[keep_cols]                # [M]
    M = la.shape[0]
    kv_order = np.argsort(la, kind="stable")
    la_s = la[kv_order]
    kvpos = kv_order
    Bm = (w_k[keep_cols] * w_in[None, :])[kv_order]   # [M, H]
    Cm = w_v[keep_cols][kv_order].astype(np.float64)  # [M, D_EMB]

    counts = np.bincount(la_s, minlength=64)
    gstart = np.concatenate([[0], np.cumsum(counts)])

    # fused weights
    Wf = (w_mix[:, :D_UP] * w_out[None, :]).astype(np.float64) @ w_up.astype(np.float64)  # [H, D_EMB]
    Mq = w_up.astype(np.float64).T @ w_up.astype(np.float64)      # [D_EMB, D_EMB]
    L = np.linalg.cholesky(Mq)                                    # M = L L^T
    VL = (Cm @ L).astype(np.float32)                              # [M, D_EMB]
    VWf = (Cm @ Wf.T).astype(np.float32)                          # [M, H]

    # per-tile windows over sorted kv
    NT = BT // QT
    win = np.empty(NT, np.int64)
    need = 0
    for g in range(NT):
        l0 = lab_q[g * QT]
        l1 = lab_q[(g + 1) * QT - 1]
        win[g] = gstart[l0]
        need = max(need, gstart[l1 + 1] - gstart[l0])
    W = max(256, int(-(-need // 128) * 128))

    # padded kv arrays so windows never go OOB
    Mp = M + W
    KT_p = np.zeros((H, Mp), np.float32)
    KT_p[:, :M] = Bm.T
    VL_p = np.zeros((Mp, D_EMB), np.float32); VL_p[:M] = VL
    VW_p = np.zeros((Mp, H), np.float32); VW_p[:M] = VWf
    la_p = np.full(Mp, -1, np.int64); la_p[:M] = la_s
    kvpos_p = np.full(Mp, -1, np.int64); kvpos_p[:M] = kvpos

    # mask bias per (window col, sorted row)
    kvi = win[:, None] + np.arange(W)[None, :]           # [NT, W]
    la_w = la_p[kvi]
    kp_w = kvpos_p[kvi]
    lab_t = lab_q.reshape(NT, QT)
    st_t = st_q.reshape(NT, QT)
    en_t = en_q.reshape(NT, QT)
    valid = ((la_w[:, None, :] == lab_t[:, :, None])
             & (kp_w[:, None, :] >= st_t[:, :, None])
             & (kp_w[:, None, :] < en_t[:, :, None]))    # [NT, QT, W]
    bias = np.where(valid, np.float32(0), np.float32(-1e30))

    Wm_x = w_mix[:, D_UP:]                               # [H, H]
    wmx_host = np.ascontiguousarray(
        Wm_x.T.reshape(HC, 128, H).transpose(1, 0, 2)
        .reshape(128, HC, MC, 128).transpose(2, 0, 1, 3)
        .reshape(MC, 128, HC * 128)).astype(BF)          # [MC, 128, HC*128]

    n_kvc = W // 128
    in_maps = []
    for c in range(NC):
        x_c = np.empty((NQT, 128, HC, QT), BF)
        kt_c = np.empty((NQT, 128, HC, W), BF)
        vl_c = np.empty((NQT, 128, n_kvc, D_EMB), BF)
        vw_c = np.empty((NQT, 128, n_kvc, H), BF)
        b_c = np.empty((NQT, 128, n_kvc, QT), BF)
        for qt in range(NQT):
            g = c * NQT + qt
            rows = slice(g * QT, (g + 1) * QT)
            w0 = win[g]
            x_c[qt] = x_sorted[rows].T.reshape(HC, 128, QT).transpose(1, 0, 2)
            kt_c[qt] = KT_p[:, w0:w0 + W].reshape(HC, 128, W).transpose(1, 0, 2)
            vl_c[qt] = VL_p[w0:w0 + W].reshape(n_kvc, 128, D_EMB).transpose(1, 0, 2)
            vw_c[qt] = VW_p[w0:w0 + W].reshape(n_kvc, 128, H).transpose(1, 0, 2)
            b_c[qt] = bias[g].T.reshape(n_kvc, 128, QT).transpose(1, 0, 2)
        in_maps.append({
            "x_in": x_c, "kt_in": kt_c, "vl_in": vl_c, "vw_in": vw_c,
            "b_in": b_c, "wmx_in": wmx_host,
        })

    nc = _get_program(W)
    import time as _time
    global LAST_EXEC_S
    _t0 = _time.time()
    LAST_RESULTS = bass_utils.run_bass_kernel_spmd(nc, in_maps,
                                                   core_ids=list(range(NC)))
    LAST_EXEC_S = _time.time() - _t0
    out_sorted = np.concatenate(
        [np.asarray(r["out_d"]).astype(np.float32).transpose(2, 0, 1).reshape(NQ, H)
         for r in LAST_RESULTS.results],
        axis=0)                                          # [BT, H]
    final = np.empty((BT, H), np.float32)
    final[perm] = out_sorted
    return final.reshape(B, T, H)
